# revision 10
# baseline (speedup 1.0000x reference)
"""GCN layer kernel for Trainium2 (Bass/Tile), data-parallel over batch.

Per core (one batch element):
    out = relu(D^-1/2 A D^-1/2 (X W^T + b))

Host-side prep per core (numpy: dtype/layout marshaling + the O(N^2) deg fold):
  - ATd = (D^-1/2 A)^T cast to bf16: A transposed (the tensor engine contracts
    over partitions, so A's contraction index must live on partitions), with
    the output-row scale D^-1/2 folded in so the PSUM drain is a pure relu.
    bf16 halves HBM traffic vs the f32 A load that bottlenecked the baseline.
  - X^T, W^T, b cast to bf16; d = deg^-1/2 as a [128, 16] f32 column table for
    the y = d * (XW^T + b) scale (deg needs full A rows, which live across all
    16 device tiles; host computes it to avoid a load/matmul barrier).
  - Output returns transposed [256, 2048] bf16; host casts + transposes back.

Device schedule (per core), paced by the 16 x 1 MB A^T tile DMA stream:
  - HWDGE loads on the SP ring: d/b/X^T/W^T first, then the A^T tiles.
  - mm1 phase: y_k = d_k * (X_k W^T + b) for all 16 k, cycling 8 PSUM regions
    across 4 banks so the PE never waits on the ACT/DVE drain round-trip
    (with only 2 regions the in-order PE queue stalls on the write-after-read
    ping-pong, which stretched every later product round). Drains alternate
    ACT / DVE. Doubles as the PE HAM warmup.
  - main matmul, transposed-output form: out^T[o, r] = sum_c y[c, o] ATd[c, r]
    with y chunks stationary and ATd the moving operand in 512-wide slices:
    per tile k just 8 matmuls of N=512 into the 8 PSUM banks (o-chunk x
    r-quarter), LDWEIGHTS hidden under the 512-col streams, one clean
    accumulation group per bank. 4 of the banks are the recycled mm1 banks.
  - tail: relu drains (alternating ACT / DVE) into a bf16 staging tile, 4 x
    256 KB output DMAs on the by-then-idle SP ring.
"""

from contextlib import ExitStack

import ml_dtypes
import numpy as np

import concourse.bacc as bacc
import concourse.mybir as mybir
import concourse.tile as tile
from concourse.bass_utils import run_bass_kernel_spmd

B = 8
N = 2048
F = 256
P = 128
NT = N // P  # 16 A^T row tiles
FT = F // P  # 2 feature tiles
RQ = 4  # 512-wide r-quarters per A^T tile
RW = N // RQ  # 512
F32 = mybir.dt.float32
BF16 = mybir.dt.bfloat16
COPY = mybir.ActivationFunctionType.Copy
RELU = mybir.ActivationFunctionType.Relu
MULT = mybir.AluOpType.mult
MAX = mybir.AluOpType.max
BF = ml_dtypes.bfloat16


def _emit(ctx: ExitStack, tc: tile.TileContext, AT, XT, WT, BIAS, DCOL, OUT):
    nc = tc.nc

    const = ctx.enter_context(tc.tile_pool(name="const", bufs=1))
    atp = ctx.enter_context(tc.tile_pool(name="atp", bufs=1))
    psum = ctx.enter_context(tc.tile_pool(name="psum", bufs=4, space="PSUM"))

    xt_sb = const.tile([P, FT * N], BF16, tag="xt")
    wt_sb = const.tile([P, FT * F], BF16, tag="wt")
    dcol = const.tile([P, NT], F32, tag="dcol")
    bias_sb = const.tile([1, F], BF16, tag="bias")
    ones1 = const.tile([1, P], BF16, tag="ones")
    y_big = const.tile([P, NT * F], BF16, tag="y")
    out_t = const.tile([P, FT * N], BF16, tag="out")
    at_big = atp.tile([P, NT * N], BF16, tag="at")

    # input DMAs (HWDGE, SP ring): mm1 operands first (xt in interleaved
    # halves so the first mm1 chains start one DMA earlier), then A^T tiles
    nc.sync.dma_start(out=dcol[:, :], in_=DCOL[:, :])
    nc.sync.dma_start(out=bias_sb[:, :], in_=BIAS[:, :])
    for phi in range(FT):
        nc.sync.dma_start(
            out=wt_sb[:, phi * F : (phi + 1) * F], in_=WT[phi * P : (phi + 1) * P, :]
        )
    HN = N // 2
    for half in range(2):
        for phi in range(FT):
            nc.sync.dma_start(
                out=xt_sb[:, phi * N + half * HN : phi * N + (half + 1) * HN],
                in_=XT[phi * P : (phi + 1) * P, half * HN : (half + 1) * HN],
            )
    for k in range(NT):
        nc.sync.dma_start(
            out=at_big[:, k * N : (k + 1) * N], in_=AT[k * P : (k + 1) * P, :]
        )

    nc.vector.memset(ones1[:, :], 1.0)

    # ---- mm1: y_k for all k through 8 regions in 4 banks ----
    mm1 = [psum.tile([P, 2 * F], F32, tag="mm1", bufs=4, name=f"mm1_{i}") for i in range(4)]

    # HAM warmup: junk 1-partition matmuls (results overwritten by mm1)
    for j in range(32):
        nc.tensor.matmul(
            mm1[j % 4][:1, :P],
            ones1[:, :1],
            ones1[:, :],
            start=True,
            stop=True,
            skip_group_check=True,
        )

    def reg_of(k):
        h = (k // 4) % 2
        return mm1[k % 4][:, h * F : (h + 1) * F]

    # 4-way interleaved chains: consecutive MMs hit different banks, so the
    # accumulate read-modify-write into the same region never stalls the PE
    for g in range(NT // 4):
        ks = range(4 * g, 4 * g + 4)
        for k in ks:
            nc.tensor.matmul(
                reg_of(k), ones1[:, :], bias_sb[:, :], start=True, stop=False,
                skip_group_check=True,
            )
        for phi in range(FT):
            for k in ks:
                nc.tensor.matmul(
                    reg_of(k),
                    xt_sb[:, phi * N + k * P : phi * N + (k + 1) * P],
                    wt_sb[:, phi * F : (phi + 1) * F],
                    start=False,
                    stop=(phi == FT - 1),
                    skip_group_check=True,
                )
        for k in ks:
            dst = y_big[:, k * F : (k + 1) * F]
            if k % 2 == 0:
                nc.scalar.activation(dst, reg_of(k), COPY, scale=dcol[:, k : k + 1])
            else:
                nc.vector.tensor_scalar(
                    out=dst, in0=reg_of(k), scalar1=dcol[:, k : k + 1],
                    scalar2=None, op0=MULT,
                )

    # ---- main matmul, transposed output: 8 banks = (o-chunk, r-quarter) ----
    banks = {}
    for oc in range(FT):
        for rc in range(0, RQ, 2):
            banks[(oc, rc)] = psum.tile(
                [P, 2 * F], F32, tag="bank", bufs=4, name=f"bank_{oc}_{rc}"
            )
    for oc in range(FT):  # recycled mm1 banks
        for rc in range(1, RQ, 2):
            banks[(oc, rc)] = psum.tile(
                [P, 2 * F], F32, tag="mm1", bufs=4, name=f"bank_{oc}_{rc}"
            )

    for k in range(NT):
        for rc in range(RQ):
            for oc in range(FT):
                for hf in range(2):
                    nc.tensor.matmul(
                        banks[(oc, rc)][:, hf * F : (hf + 1) * F],
                        y_big[:, k * F + oc * P : k * F + (oc + 1) * P],
                        at_big[
                            :,
                            k * N + rc * RW + hf * F : k * N + rc * RW + (hf + 1) * F,
                        ],
                        start=(k == 0 and hf == 0),
                        stop=(k == NT - 1),
                        skip_group_check=True,
                    )

    # ---- drains (pure relu; d_r folded into ATd) + 4 output DMAs ----
    for oc in range(FT):
        for rc in range(RQ):
            src = banks[(oc, rc)][:, :RW]
            dst = out_t[:, oc * N + rc * RW : oc * N + (rc + 1) * RW]
            if rc % 2 == 0:
                nc.scalar.activation(dst, src, RELU)
            else:
                nc.vector.tensor_scalar(
                    out=dst, in0=src, scalar1=0.0, scalar2=None, op0=MAX
                )
            if rc % 2 == 1:
                half = rc // 2
                nc.sync.dma_start(
                    out=OUT[oc * P : (oc + 1) * P, half * 2 * RW : (half + 1) * 2 * RW],
                    in_=out_t[
                        :, oc * N + half * 2 * RW : oc * N + (half + 1) * 2 * RW
                    ],
                )


_cached_nc = None


def _build():
    nc = bacc.Bacc("TRN2", target_bir_lowering=False, debug=False)
    AT = nc.dram_tensor("at", [N, N], BF16, kind="ExternalInput").ap()
    XT = nc.dram_tensor("xt", [F, N], BF16, kind="ExternalInput").ap()
    WT = nc.dram_tensor("wt", [F, F], BF16, kind="ExternalInput").ap()
    BIAS = nc.dram_tensor("bias", [1, F], BF16, kind="ExternalInput").ap()
    DCOL = nc.dram_tensor("dcol", [P, NT], F32, kind="ExternalInput").ap()
    OUT = nc.dram_tensor("out", [F, N], BF16, kind="ExternalOutput").ap()
    with tile.TileContext(nc) as tc:
        with ExitStack() as ctx:
            _emit(ctx, tc, AT, XT, WT, BIAS, DCOL, OUT)
    nc.compile()
    return nc


def get_nc():
    global _cached_nc
    if _cached_nc is None:
        _cached_nc = _build()
    return _cached_nc


def make_in_maps(node_features, adj_matrix, W, b):
    node_features = np.asarray(node_features, dtype=np.float32)
    adj_matrix = np.asarray(adj_matrix, dtype=np.float32)
    wt = np.ascontiguousarray(np.asarray(W, dtype=np.float32).T.astype(BF))
    bias = np.ascontiguousarray(
        np.asarray(b, dtype=np.float32).astype(BF).reshape(1, F)
    )
    maps = []
    for c in range(B):
        adj = adj_matrix[c]
        deg = adj.sum(axis=1, dtype=np.float32)
        with np.errstate(divide="ignore"):
            d = deg**-0.5
        d = np.where(np.isfinite(d), d, 0.0).astype(np.float32)
        maps.append(
            {
                # (D^-1/2 A)^T: row scale folded in before the bf16 cast
                "at": np.ascontiguousarray((adj * d[:, None]).astype(BF).T),
                "xt": np.ascontiguousarray(node_features[c].T.astype(BF)),
                "wt": wt,
                "bias": bias,
                "dcol": np.ascontiguousarray(d.reshape(NT, P).T),
            }
        )
    return maps


def unpack_out(arr):
    """Device output [F, N] bf16 -> full-precision [N, F] f32."""
    return np.ascontiguousarray(np.asarray(arr).astype(np.float32).T)


def kernel(node_features, adj_matrix, W, b):
    nc = get_nc()
    in_maps = make_in_maps(node_features, adj_matrix, W, b)
    res = run_bass_kernel_spmd(nc, in_maps, core_ids=list(range(B)))
    return np.stack([unpack_out(r["out"]) for r in res.results], axis=0)


# revision 11
# speedup vs baseline: 1.0903x; 1.0903x over previous
"""GCN layer kernel for Trainium2 (Bass/Tile), data-parallel over batch.

Per core (one batch element):
    out = relu(D^-1/2 A D^-1/2 (X W^T + b))

Decomposition: with d = deg^-1/2, Ad = (D^-1/2 A D^-1/2)  (both scales folded),
    out^T[o, r] = relu( sum_c Ad^T[c, r] * (Xd W^T)[c, o]  +  b[o] * v[r] )
where Xd = D^-1/2 X and v = Ad @ d... precisely v[r] = d_r * (A @ d)[r], so the
bias contribution is a rank-1 term that enters PSUM as a 1-partition matmul.

Host-side prep per core (numpy: layout/dtype marshaling + O(N^2) reductions):
  - ATd = (D^-1/2 A)^T bf16 (transposed: the tensor engine contracts over
    partitions; bf16 halves the HBM traffic that bottlenecked the baseline),
  - Xd^T bf16, W^T bf16, b bf16, v = d*(A@d) bf16,
  - deg/d computed on host: deg needs full A rows, which live across all 16
    device tiles of ATd; on device it would serialize loads vs compute.
  - Output returns transposed [256, 2048] bf16; host casts + transposes back.

Device schedule (per core) — the PE stream is the critical path, so every
stage is shaped to stream at the 109 ns/matmul issue floor:
  - HWDGE loads on the SP ring: b/v/W^T first, Xd^T in two half-loads, then
    16 x 512 KB ATd tiles.
  - mm1 (y = Xd W^T): 16 two-matmul accumulation chains, interleaved 8-way
    across all 8 PSUM banks so the same-region read-modify-write never stalls
    the in-order PE queue; drains alternate ACT/DVE. Doubles as HAM warmup.
  - rank-1 bias init: 16 one-partition matmuls b_chunk x v_slice open the 8
    product banks (start=True once per bank; the second half-region's first
    write lands on cleared has_written bits and overwrites).
  - main matmul: out^T accumulates per arriving ATd tile k: 16 products of
    N=256 (512-col moving would disable the LDWEIGHTS pull-ahead and run
    ~4.5x slower - measured), stationary = y chunk, moving = ATd slice.
  - tail: pure-relu drains (alternating ACT / DVE), 4 x 256 KB output DMAs.
"""

from contextlib import ExitStack

import ml_dtypes
import numpy as np

import concourse.bacc as bacc
import concourse.mybir as mybir
import concourse.tile as tile
from concourse.bass_utils import run_bass_kernel_spmd

B = 8
N = 2048
F = 256
P = 128
NT = N // P  # 16 A^T row tiles
FT = F // P  # 2 feature tiles
RQ = 4  # r-quarters (one PSUM bank each)
RW = N // RQ  # 512
F32 = mybir.dt.float32
BF16 = mybir.dt.bfloat16
COPY = mybir.ActivationFunctionType.Copy
RELU = mybir.ActivationFunctionType.Relu
MULT = mybir.AluOpType.mult
ADD = mybir.AluOpType.add
MAX = mybir.AluOpType.max
BF = ml_dtypes.bfloat16

HOST_MM1 = False  # ship y precomputed on host instead of mm1 on device


def _emit(ctx: ExitStack, tc: tile.TileContext, nc, AT, XT, WT, BIAS, V, OUT):
    const = ctx.enter_context(tc.tile_pool(name="const", bufs=1))
    atp = ctx.enter_context(tc.tile_pool(name="atp", bufs=1))
    psum = ctx.enter_context(tc.tile_pool(name="psum", bufs=8, space="PSUM"))

    bias_sb = const.tile([1, F], BF16, tag="bias")
    v_sb = const.tile([1, N], BF16, tag="v")
    y_big = const.tile([P, NT * F], BF16, tag="y")
    out_t = const.tile([P, FT * N], BF16, tag="out")
    at_big = atp.tile([P, NT * N], BF16, tag="at")

    # input DMAs (HWDGE, SP ring): small mm1 operands first, then ATd tiles
    nc.sync.dma_start(out=bias_sb[:, :], in_=BIAS[:, :])
    nc.sync.dma_start(out=v_sb[:, :], in_=V[:, :])
    if HOST_MM1:
        nc.sync.dma_start(
            out=y_big[:, :].rearrange("p (k o) -> p k o", k=NT),
            in_=XT.rearrange("(k p) o -> p k o", p=P),
        )
    else:
        xt_sb = const.tile([P, FT * N], BF16, tag="xt")
        wt_sb = const.tile([P, FT * F], BF16, tag="wt")
        for phi in range(FT):
            nc.sync.dma_start(
                out=wt_sb[:, phi * F : (phi + 1) * F],
                in_=WT[phi * P : (phi + 1) * P, :],
            )
        HN = N // 2
        for half in range(2):
            for phi in range(FT):
                nc.sync.dma_start(
                    out=xt_sb[:, phi * N + half * HN : phi * N + (half + 1) * HN],
                    in_=XT[phi * P : (phi + 1) * P, half * HN : (half + 1) * HN],
                )
    for k in range(NT):
        nc.sync.dma_start(
            out=at_big[:, k * N : (k + 1) * N], in_=AT[k * P : (k + 1) * P, :]
        )

    # ---- mm1: y = Xd @ W^T, 16 2-chains interleaved 8-way over all banks ----
    if not HOST_MM1:
        mm1t = [
            psum.tile([P, 2 * F], F32, tag="bank", name=f"mm1_{i}") for i in range(8)
        ]

        def reg_of(k):
            h = k // 8
            return mm1t[k % 8][:, h * F : (h + 1) * F]

        for blk in range(2):
            ks = range(8 * blk, 8 * blk + 8)
            for phi in range(FT):
                for k in ks:
                    nc.tensor.matmul(
                        reg_of(k),
                        xt_sb[:, phi * N + k * P : phi * N + (k + 1) * P],
                        wt_sb[:, phi * F : (phi + 1) * F],
                        start=(phi == 0),
                        stop=(phi == FT - 1),
                        skip_group_check=True,
                    )
            for k in ks:
                dst = y_big[:, k * F : (k + 1) * F]
                if k % 2 == 0:
                    nc.scalar.activation(dst, reg_of(k), COPY)
                else:
                    nc.vector.tensor_scalar(
                        out=dst, in0=reg_of(k), scalar1=0.0, scalar2=None, op0=ADD
                    )

    # ---- product banks + rank-1 bias init: psum = b ⊗ v ----
    banks = {}
    for oc in range(FT):
        for rc in range(RQ):
            banks[(oc, rc)] = psum.tile(
                [P, 2 * F], F32, tag="bank", name=f"bank_{oc}_{rc}"
            )
    for oc in range(FT):
        for rc in range(RQ):
            for hf in range(2):
                nc.tensor.matmul(
                    banks[(oc, rc)][:, hf * F : (hf + 1) * F],
                    bias_sb[:, oc * P : (oc + 1) * P],
                    v_sb[:, rc * RW + hf * F : rc * RW + (hf + 1) * F],
                    start=(hf == 0),
                    stop=False,
                    skip_group_check=True,
                )

    # ---- main matmul: out^T[o, r] += sum_c y[c, o] ATd[c, r] ----
    for k in range(NT):
        for rc in range(RQ):
            for oc in range(FT):
                for hf in range(2):
                    nc.tensor.matmul(
                        banks[(oc, rc)][:, hf * F : (hf + 1) * F],
                        y_big[:, k * F + oc * P : k * F + (oc + 1) * P],
                        at_big[
                            :,
                            k * N + rc * RW + hf * F : k * N + rc * RW + (hf + 1) * F,
                        ],
                        start=False,
                        stop=(k == NT - 1),
                        skip_group_check=True,
                    )

    # ---- drains (pure relu) + 4 output DMAs ----
    for oc in range(FT):
        for rc in range(RQ):
            src = banks[(oc, rc)][:, : 2 * F]
            dst = out_t[:, oc * N + rc * RW : oc * N + (rc + 1) * RW]
            if rc % 2 == 0:
                nc.scalar.activation(dst, src, RELU)
            else:
                nc.vector.tensor_scalar(
                    out=dst, in0=src, scalar1=0.0, scalar2=None, op0=MAX
                )
            if rc % 2 == 1:
                half = rc // 2
                nc.sync.dma_start(
                    out=OUT[oc * P : (oc + 1) * P, half * 2 * RW : (half + 1) * 2 * RW],
                    in_=out_t[
                        :, oc * N + half * 2 * RW : oc * N + (half + 1) * 2 * RW
                    ],
                )


_cached_nc = None


def _build():
    nc = bacc.Bacc("TRN2", target_bir_lowering=False, debug=False)
    AT = nc.dram_tensor("at", [N, N], BF16, kind="ExternalInput").ap()
    xt_shape = [N, F] if HOST_MM1 else [F, N]
    XT = nc.dram_tensor("xt", xt_shape, BF16, kind="ExternalInput").ap()
    WT = nc.dram_tensor("wt", [F, F], BF16, kind="ExternalInput").ap()
    BIAS = nc.dram_tensor("bias", [1, F], BF16, kind="ExternalInput").ap()
    V = nc.dram_tensor("v", [1, N], BF16, kind="ExternalInput").ap()
    OUT = nc.dram_tensor("out", [F, N], BF16, kind="ExternalOutput").ap()
    with tile.TileContext(nc) as tc:
        with ExitStack() as ctx:
            _emit(ctx, tc, nc, AT, XT, WT, BIAS, V, OUT)
    nc.compile()
    return nc


def get_nc():
    global _cached_nc
    if _cached_nc is None:
        _cached_nc = _build()
    return _cached_nc


def make_in_maps(node_features, adj_matrix, W, b):
    node_features = np.asarray(node_features, dtype=np.float32)
    adj_matrix = np.asarray(adj_matrix, dtype=np.float32)
    W = np.asarray(W, dtype=np.float32)
    b32 = np.asarray(b, dtype=np.float32)
    wt = np.ascontiguousarray(W.T.astype(BF))
    bias = np.ascontiguousarray(b32.astype(BF).reshape(1, F))
    maps = []
    for c in range(B):
        adj = adj_matrix[c]
        deg = adj.sum(axis=1, dtype=np.float32)
        with np.errstate(divide="ignore"):
            d = deg**-0.5
        d = np.where(np.isfinite(d), d, 0.0).astype(np.float32)
        xd = node_features[c] * d[:, None]  # D^-1/2 X
        if HOST_MM1:
            xt = np.ascontiguousarray((xd @ W.T).astype(BF))  # y rows
        else:
            xt = np.ascontiguousarray(xd.T.astype(BF))
        maps.append(
            {
                # (D^-1/2 A)^T: output-row scale folded in before the bf16 cast
                "at": np.ascontiguousarray((adj * d[:, None]).astype(BF).T),
                "xt": xt,
                "wt": wt,
                "bias": bias,
                "v": np.ascontiguousarray((d * (adj @ d)).astype(BF).reshape(1, N)),
            }
        )
    return maps


def unpack_out(arr):
    """Device output [F, N] bf16 -> full-precision [N, F] f32."""
    return np.ascontiguousarray(np.asarray(arr).astype(np.float32).T)


def kernel(node_features, adj_matrix, W, b):
    nc = get_nc()
    in_maps = make_in_maps(node_features, adj_matrix, W, b)
    res = run_bass_kernel_spmd(nc, in_maps, core_ids=list(range(B)))
    return np.stack([unpack_out(r["out"]) for r in res.results], axis=0)


# revision 13
# speedup vs baseline: 1.0989x; 1.0079x over previous
"""GCN layer kernel for Trainium2 (Bass/Tile), data-parallel over batch.

Per core (one batch element):
    out = relu(D^-1/2 A D^-1/2 (X W^T + b))

Decomposition: with d = deg^-1/2,
    out^T[o, r] = relu( sum_c ATd[c, r] * y[c, o]  +  b[o] * v[r] )
where ATd = (D^-1/2 A)^T, y = (D^-1/2 X) W^T, v[r] = d_r * (A @ d)[r]: both
diagonal scales are folded into the operands and the bias becomes a rank-1
term entering PSUM as 1-partition matmuls, so the drain is a pure relu.

Host-side prep per core (numpy: layout/dtype marshaling + O(N^2) reductions):
ATd bf16 (transposed: the tensor engine contracts over partitions; bf16 halves
the HBM traffic that bottlenecked the baseline), Xd^T bf16 packed [128, 4096],
W^T packed [128, 512], bias|v packed [1, 2304]. deg/d on host: deg needs full
A rows, which live across all 16 device tiles of ATd. Output returns
transposed [256, 2048] bf16; host casts + transposes back.

Device schedule (per core) — the PE stream is the critical path (the 256
N=256 products are at the bf16 roofline), so everything is shaped to stream
at the 109 ns/matmul issue floor with zero in-order-queue stalls:
  - 3 packed small HWDGE loads (each dma_start costs ~0.6us of serial issue
    time on the SP sequencer - measured), then 16 x 512 KB ATd tiles.
  - mm1 (y = Xd W^T): 16 two-matmul accumulation chains interleaved 8-way
    across all 8 PSUM banks so the same-region read-modify-write RAW never
    stalls the PE; drains alternate ACT/DVE. Doubles as HAM warmup.
  - rank-1 bias init: 16 one-partition matmuls b_chunk x v_slice open the 8
    recycled product banks (start=True once per bank clears has_written; the
    second half-region's first write lands on cleared bits and overwrites).
  - main matmul: out^T accumulates per arriving ATd tile k: 16 products of
    N=256 (512-col moving disables the LDWEIGHTS pull-ahead and runs ~4.5x
    slower - measured), stationary = y chunk, moving = ATd slice.
  - last round interleaves per-bank pure-relu drains (alternating ACT / DVE)
    and 4 x 256 KB output DMAs split across the SP and ACT HWDGE rings.
"""

from contextlib import ExitStack

import ml_dtypes
import numpy as np

import concourse.bacc as bacc
import concourse.mybir as mybir
import concourse.tile as tile
from concourse.bass_utils import run_bass_kernel_spmd

B = 8
N = 2048
F = 256
P = 128
NT = N // P  # 16 A^T row tiles
FT = F // P  # 2 feature tiles
RQ = 4  # r-quarters (one PSUM bank each)
RW = N // RQ  # 512
HN = N // 2
F32 = mybir.dt.float32
BF16 = mybir.dt.bfloat16
COPY = mybir.ActivationFunctionType.Copy
RELU = mybir.ActivationFunctionType.Relu
ADD = mybir.AluOpType.add
MAX = mybir.AluOpType.max
BF = ml_dtypes.bfloat16

HOST_MM1 = False  # ship y precomputed on host instead of mm1 on device


def _emit(ctx: ExitStack, tc: tile.TileContext, nc, AT, XT, WT, BV, OUT):
    const = ctx.enter_context(tc.tile_pool(name="const", bufs=1))
    atp = ctx.enter_context(tc.tile_pool(name="atp", bufs=1))
    psum = ctx.enter_context(tc.tile_pool(name="psum", bufs=8, space="PSUM"))

    bv_sb = const.tile([1, F + N], BF16, tag="bv")
    y_big = const.tile([P, NT * F], BF16, tag="y")
    out_t = const.tile([P, FT * N], BF16, tag="out")
    at_big = atp.tile([P, NT * N], BF16, tag="at")
    bias_ap = bv_sb[:, :F]
    v_ap = bv_sb[:, F:]

    # input DMAs (HWDGE, SP ring): packed small tensors first, then ATd tiles
    nc.sync.dma_start(out=bv_sb[:, :], in_=BV[:, :])
    if HOST_MM1:
        nc.sync.dma_start(
            out=y_big[:, :].rearrange("p (k o) -> p k o", k=NT),
            in_=XT.rearrange("(k p) o -> p k o", p=P),
        )
    else:
        wt_sb = const.tile([P, FT * F], BF16, tag="wt")
        xt_sb = const.tile([P, FT * N], BF16, tag="xt")
        nc.sync.dma_start(out=wt_sb[:, :], in_=WT[:, :])
        nc.sync.dma_start(out=xt_sb[:, :], in_=XT[:, :])
    for k in range(NT):
        nc.sync.dma_start(
            out=at_big[:, k * N : (k + 1) * N], in_=AT[k * P : (k + 1) * P, :]
        )

    # ---- mm1: y = Xd @ W^T, 16 2-chains interleaved 8-way over all banks ----
    if not HOST_MM1:
        mm1t = [
            psum.tile([P, 2 * F], F32, tag="bank", name=f"mm1_{i}") for i in range(8)
        ]

        def reg_of(k):
            return mm1t[k % 8][:, (k // 8) * F : (k // 8 + 1) * F]

        def xt_chunk(k, phi):
            # packed layout: [half(1024) x phi(1024)] blocks, half = k // 8
            base = (2 * (k // 8) + phi) * HN
            return xt_sb[:, base + (k % 8) * P : base + (k % 8 + 1) * P]

        for blk in range(2):
            ks = range(8 * blk, 8 * blk + 8)
            for phi in range(FT):
                for k in ks:
                    nc.tensor.matmul(
                        reg_of(k),
                        xt_chunk(k, phi),
                        wt_sb[:, phi * F : (phi + 1) * F],
                        start=(phi == 0),
                        stop=(phi == FT - 1),
                        skip_group_check=True,
                    )
            for k in ks:
                dst = y_big[:, k * F : (k + 1) * F]
                if k % 2 == 0:
                    nc.scalar.activation(dst, reg_of(k), COPY)
                else:
                    nc.vector.tensor_scalar(
                        out=dst, in0=reg_of(k), scalar1=0.0, scalar2=None, op0=ADD
                    )

    # ---- product banks + rank-1 bias init: psum = b ⊗ v ----
    banks = {}
    for oc in range(FT):
        for rc in range(RQ):
            banks[(oc, rc)] = psum.tile(
                [P, 2 * F], F32, tag="bank", name=f"bank_{oc}_{rc}"
            )
    for oc in range(FT):
        for rc in range(RQ):
            for hf in range(2):
                nc.tensor.matmul(
                    banks[(oc, rc)][:, hf * F : (hf + 1) * F],
                    bias_ap[:, oc * P : (oc + 1) * P],
                    v_ap[:, rc * RW + hf * F : rc * RW + (hf + 1) * F],
                    start=(hf == 0),
                    stop=False,
                    skip_group_check=True,
                )

    # ---- main matmul: out^T[o, r] += sum_c y[c, o] ATd[c, r] ----
    def emit_product(k, rc, oc, hf):
        nc.tensor.matmul(
            banks[(oc, rc)][:, hf * F : (hf + 1) * F],
            y_big[:, k * F + oc * P : k * F + (oc + 1) * P],
            at_big[:, k * N + rc * RW + hf * F : k * N + rc * RW + (hf + 1) * F],
            start=False,
            stop=(k == NT - 1),
            skip_group_check=True,
        )

    for k in range(NT - 1):
        for rc in range(RQ):
            for oc in range(FT):
                for hf in range(2):
                    emit_product(k, rc, oc, hf)

    # ---- last round: interleave products, pure-relu drains, output DMAs ----
    for oc in range(FT):
        for rc in range(RQ):
            for hf in range(2):
                emit_product(NT - 1, rc, oc, hf)
            src = banks[(oc, rc)][:, : 2 * F]
            dst = out_t[:, oc * N + rc * RW : oc * N + (rc + 1) * RW]
            if rc % 2 == 0:
                nc.scalar.activation(dst, src, RELU)
            else:
                nc.vector.tensor_scalar(
                    out=dst, in0=src, scalar1=0.0, scalar2=None, op0=MAX
                )
            if rc % 2 == 1:
                half = rc // 2
                eng = nc.sync if half == 0 else nc.scalar
                eng.dma_start(
                    out=OUT[oc * P : (oc + 1) * P, half * 2 * RW : (half + 1) * 2 * RW],
                    in_=out_t[
                        :, oc * N + half * 2 * RW : oc * N + (half + 1) * 2 * RW
                    ],
                )


_cached_nc = None


def _build():
    nc = bacc.Bacc("TRN2", target_bir_lowering=False, debug=False)
    AT = nc.dram_tensor("at", [N, N], BF16, kind="ExternalInput").ap()
    xt_shape = [N, F] if HOST_MM1 else [P, FT * N]
    XT = nc.dram_tensor("xt", xt_shape, BF16, kind="ExternalInput").ap()
    WT = nc.dram_tensor("wt", [P, FT * F], BF16, kind="ExternalInput").ap()
    BV = nc.dram_tensor("bv", [1, F + N], BF16, kind="ExternalInput").ap()
    OUT = nc.dram_tensor("out", [F, N], BF16, kind="ExternalOutput").ap()
    with tile.TileContext(nc) as tc:
        with ExitStack() as ctx:
            _emit(ctx, tc, nc, AT, XT, WT, BV, OUT)
    nc.compile()
    return nc


def get_nc():
    global _cached_nc
    if _cached_nc is None:
        _cached_nc = _build()
    return _cached_nc


def make_in_maps(node_features, adj_matrix, W, b):
    node_features = np.asarray(node_features, dtype=np.float32)
    adj_matrix = np.asarray(adj_matrix, dtype=np.float32)
    W = np.asarray(W, dtype=np.float32)
    b32 = np.asarray(b, dtype=np.float32)
    wt_bf = W.T.astype(BF)  # [f, o]
    # packed [128, 512]: wt[p, phi*F + o] = W.T[phi*128 + p, o]
    wt = np.ascontiguousarray(
        np.concatenate([wt_bf[phi * P : (phi + 1) * P, :] for phi in range(FT)], axis=1)
    )
    maps = []
    for c in range(B):
        adj = adj_matrix[c]
        deg = adj.sum(axis=1, dtype=np.float32)
        with np.errstate(divide="ignore"):
            d = deg**-0.5
        d = np.where(np.isfinite(d), d, 0.0).astype(np.float32)
        xd = node_features[c] * d[:, None]  # D^-1/2 X
        if HOST_MM1:
            xt = np.ascontiguousarray((xd @ W.T).astype(BF))  # y rows
        else:
            xdt = xd.T.astype(BF)  # [f, m]
            # packed [128, 4096]: [half x phi] blocks of [128, 1024]
            xt = np.ascontiguousarray(
                np.concatenate(
                    [
                        xdt[phi * P : (phi + 1) * P, half * HN : (half + 1) * HN]
                        for half in range(2)
                        for phi in range(FT)
                    ],
                    axis=1,
                )
            )
        v = (d * (adj @ d)).astype(BF)
        bv = np.ascontiguousarray(
            np.concatenate([b32.astype(BF).reshape(1, F), v.reshape(1, N)], axis=1)
        )
        maps.append(
            {
                # (D^-1/2 A)^T: output-row scale folded in before the bf16 cast
                "at": np.ascontiguousarray((adj * d[:, None]).astype(BF).T),
                "xt": xt,
                "wt": wt,
                "bv": bv,
            }
        )
    return maps


def unpack_out(arr):
    """Device output [F, N] bf16 -> full-precision [N, F] f32."""
    return np.ascontiguousarray(np.asarray(arr).astype(np.float32).T)


def kernel(node_features, adj_matrix, W, b):
    nc = get_nc()
    in_maps = make_in_maps(node_features, adj_matrix, W, b)
    res = run_bass_kernel_spmd(nc, in_maps, core_ids=list(range(B)))
    return np.stack([unpack_out(r["out"]) for r in res.results], axis=0)
